# revision 8
# baseline (speedup 1.0000x reference)
"""Block-circulant linear (MINI_BLOCK=4) via length-4 rFFT factorization on 8 trn2 cores.

Math: out = x @ W^T where W[4y+n, 4x+j] = eigens[y, x, (n-j) mod 4].
In the length-4 DFT domain the circulant contraction factors into 5 real
matmul chains over the block-index axis gx=1024 (Gauss 3-mult for the complex
bin; ~3.2x fewer FLOPs than the dense 4096^3 matmul):
  X0 = x0+x1+x2+x3, X1 = (x0-x2) + i(x3-x1), X2 = x0-x1+x2-x3  (per block of 4)
  Y0 = X0 E0, Y2 = X2 E2
  Gauss (combos on the X side so only ONE derived E matrix is needed):
    g1 = X1r(E1r+E1i), g2 = (X1i-X1r)E1r, g3 = (X1r+X1i)E1i
    Y1r = g1-g3, Y1i = g1+g2
  o0 = Y0+Y1r+Y2, o1 = Y0-Y1i-Y2, o2 = Y0-Y1r+Y2, o3 = Y0+Y1i-Y2  (scales in E)

Device does ONLY the five matmul chains (the FLOP core) plus cheap DVE adds;
both DFT butterflies are data-independent linear prep and run on the host.
The kernel is ramp-bound at the start (tile0 needs E-yc0 + all of x before
its last chain matmul; HBM sustains ~340 GB/s/core), so transport is
minimized: 4 x-streams (s02,s13,X1r,X1i; the 5 matmul streams are derived
on-device by the otherwise-idle DVE) and 4 E matrices (Es=E1r+E1i derived
on-device): 8 MB ramp-critical, 12 MB total in, 4 MB (bf16 Y-streams) out.
Tensor engine: 320x 128x128x512 bf16 matmuls at 216 ns sustained = 69 us.

Sharding: data-parallel over batch, 512 rows per core; E replicated.
bf16 operands, fp32 PSUM; Y-streams returned bf16 (adds ~5e-4 rel err).
"""
import numpy as np

B, IN, OUT, BLK = 4096, 4096, 4096, 4
GX, GY = IN // BLK, OUT // BLK        # 1024, 1024
NCORES = 8
BS = B // NCORES                      # 512 batch rows per core
BT = BS // 128                        # 4 b-tiles
XC = GX // 128                        # 8 x-chunks (contraction)
YCS = 512                             # y-chunk size (matmul N)
YCN = GY // YCS                       # 2 y-chunks

_cache = {}


def _build_nc():
    from concourse import bacc
    import concourse.mybir as mybir
    from concourse.tile import TileContext

    f32 = mybir.dt.float32
    bf16 = mybir.dt.bfloat16

    nc = bacc.Bacc("TRN2", target_bir_lowering=False, debug=False,
                   enable_asserts=False, num_devices=NCORES)
    # 4 host-prepped x streams (s02, s13, X1r, X1i), transposed [s, gx, b]
    xs_d = nc.dram_tensor("xs", [4, GX, BS], bf16, kind="ExternalInput")
    # 4 E matrices; Es = E1r+E1i is derived on-device
    e_d = [nc.dram_tensor(nm, [YCN, XC, 128, YCS], bf16, kind="ExternalInput")
           for nm in ("e0", "e1r", "e1i", "e2")]
    # 4 Y streams out: Y0, Y2, Y1r, Y1i (host applies the inverse butterfly)
    ys_d = nc.dram_tensor("ys", [4, BS, GY], bf16, kind="ExternalOutput")

    with TileContext(nc) as tc:
        with (
            tc.tile_pool(name="xt", bufs=1) as xtp,
            tc.tile_pool(name="epool", bufs=2) as ep,
            tc.tile_pool(name="vpool", bufs=2) as vp,
            tc.tile_pool(name="outp", bufs=3) as op_,
            tc.tile_pool(name="mpsum", bufs=1, space="PSUM") as mps,
        ):
            # DMA-landed x streams
            xin = [xtp.tile([128, XC, BS], bf16, tag=f"xin{s}", name=f"xin{s}")
                   for s in range(4)]  # s02, s13, X1r, X1i
            # derived matmul streams (X1r used straight from xin[2])
            x0t = xtp.tile([128, XC, BS], bf16, tag="x0t", name="x0t")   # X0
            x2t = xtp.tile([128, XC, BS], bf16, tag="x2t", name="x2t")   # X2
            xdt = xtp.tile([128, XC, BS], bf16, tag="xdt", name="xdt")   # X1i-X1r
            xst = xtp.tile([128, XC, BS], bf16, tag="xst", name="xst")   # X1r+X1i

            def e_tiles():
                t = [ep.tile([128, XC, YCS], bf16, tag=f"e{k}", name=f"et{k}")
                     for k in range(5)]  # E0, E1r, E1i, E2, Es(derived)
                return t

            def e_half(et, yc, k, h, eng):
                hs = slice(4 * h, 4 * h + 4)
                eng.dma_start(out=et[k][:, hs],
                              in_=e_d[k][yc].rearrange("c p y -> p c y")[:, hs])
                if k == 2:  # Es = E1r + E1i once both halves present
                    nc.vector.tensor_add(out=et[4][:, hs], in0=et[1][:, hs],
                                         in1=et[2][:, hs])

            et0 = e_tiles()
            xsv = [xs_d[s].rearrange("(c p) b -> c p b", p=128) for s in range(4)]
            # Ramp: tile0 needs E-yc0 + all of x (8 MB) and the ramp is
            # HBM-bound (~330 GB/s/core aggregate), so interleave x chunks
            # xc-major round-robin across all three queues with the E halves
            # placed so each chain's operands land in consumption order
            # (chains run y0, y2, g2, g3, g1).
            x1i_eng = [nc.sync, nc.scalar, nc.gpsimd]
            for xc in range(XC):
                nc.sync.dma_start(out=xin[0][:, xc], in_=xsv[0][xc])
                nc.gpsimd.dma_start(out=xin[1][:, xc], in_=xsv[1][xc])
                nc.scalar.dma_start(out=xin[2][:, xc], in_=xsv[2][xc])
                x1i_eng[xc % 3].dma_start(out=xin[3][:, xc], in_=xsv[3][xc])
                if xc == 0:
                    e_half(et0, 0, 0, 0, nc.gpsimd)
                    e_half(et0, 0, 3, 0, nc.sync)
                    e_half(et0, 0, 1, 0, nc.scalar)
                elif xc == 1:
                    e_half(et0, 0, 2, 0, nc.gpsimd)
                elif xc == 3:
                    e_half(et0, 0, 0, 1, nc.gpsimd)
                    e_half(et0, 0, 3, 1, nc.sync)
                    e_half(et0, 0, 1, 1, nc.scalar)
                elif xc == 4:
                    e_half(et0, 0, 2, 1, nc.sync)
                # forward butterfly (bf16, on the otherwise-idle DVE)
                nc.vector.tensor_add(out=x0t[:, xc], in0=xin[0][:, xc], in1=xin[1][:, xc])
                nc.vector.tensor_sub(out=x2t[:, xc], in0=xin[0][:, xc], in1=xin[1][:, xc])
                nc.vector.tensor_sub(out=xdt[:, xc], in0=xin[3][:, xc], in1=xin[2][:, xc])
                nc.vector.tensor_add(out=xst[:, xc], in0=xin[2][:, xc], in1=xin[3][:, xc])

            # Main: 5 matmul chains per (yc, bt), Gauss combine, store streams
            for yc in range(YCN):
                if yc == 0:
                    et = et0
                else:
                    et = e_tiles()
                    for h in (0, 1):
                        e_half(et, yc, 0, h, nc.gpsimd)
                        e_half(et, yc, 3, h, nc.sync)
                        e_half(et, yc, 1, h, nc.scalar)
                        e_half(et, yc, 2, h, nc.gpsimd)
                for bt in range(BT):
                    bsl = slice(bt * 128, (bt + 1) * 128)
                    # chains: y0=X0*E0, y2=X2*E2, g2=Xd*E1r, g3=X1s*E1i,
                    # g1=X1r*Es (Es derived on-device, so g1 runs last).
                    # Round-robin over PSUM banks; bufs sized so each bank is
                    # drained before the next tile's chain-start needs it.
                    y0 = mps.tile([128, YCS], f32, tag="y0")
                    y2 = mps.tile([128, YCS], f32, tag="y2", bufs=2)
                    g2 = mps.tile([128, YCS], f32, tag="g2", bufs=2)
                    g3 = mps.tile([128, YCS], f32, tag="g3", bufs=2)
                    g1 = mps.tile([128, YCS], f32, tag="g1")
                    for xc in range(XC):
                        st, sp = xc == 0, xc == XC - 1
                        nc.tensor.matmul(y0, x0t[:, xc, bsl], et[0][:, xc], start=st, stop=sp)
                        nc.tensor.matmul(y2, x2t[:, xc, bsl], et[3][:, xc], start=st, stop=sp)
                        nc.tensor.matmul(g2, xdt[:, xc, bsl], et[1][:, xc], start=st, stop=sp)
                        nc.tensor.matmul(g3, xst[:, xc, bsl], et[2][:, xc], start=st, stop=sp)
                        nc.tensor.matmul(g1, xin[2][:, xc, bsl], et[4][:, xc], start=st, stop=sp)
                    # Drain: DVE/ACT read at most ONE PSUM operand per op; g1
                    # staged via SBUF. DVE does the Gauss combine, scalar the
                    # plain copies; out-DMA pairs on sync (y0,y2) and the
                    # otherwise-idle gpsimd (Y1r,Y1i).
                    v_ = vp.tile([128, YCS], f32, tag="v")
                    ol = op_.tile([128, 2, YCS], bf16, tag="ol")
                    oh = op_.tile([128, 2, YCS], bf16, tag="oh")
                    nc.scalar.copy(out=ol[:, 0], in_=y0)                 # frees y0
                    nc.scalar.copy(out=ol[:, 1], in_=y2)                 # frees y2
                    nc.vector.tensor_copy(out=v_, in_=g1)                # frees g1
                    nc.vector.tensor_sub(out=oh[:, 0], in0=v_, in1=g3)   # Y1r, frees g3
                    nc.vector.tensor_add(out=oh[:, 1], in0=v_, in1=g2)   # Y1i, frees g2
                    ysl = ys_d[:, bsl, yc * YCS:(yc + 1) * YCS]
                    nc.sync.dma_start(
                        out=ysl[0:2].rearrange("s p y -> p s y"), in_=ol)
                    nc.gpsimd.dma_start(
                        out=ysl[2:4].rearrange("s p y -> p s y"), in_=oh)
    nc.compile()
    return nc


def _prep_eigens(eigens):
    """eigens (gy, gx, 4) -> four (YCN, XC, 128, YCS) bf16 chunked E-matrices
    (E0, E1r, E1i, E2), transposed to [x, y] with irfft scales folded in."""
    e = np.ascontiguousarray(eigens.transpose(1, 0, 2)).astype(np.float32)  # (x, y, j)
    e0 = ((e[..., 0] + e[..., 2]) + (e[..., 1] + e[..., 3])) * 0.25
    e2 = ((e[..., 0] + e[..., 2]) - (e[..., 1] + e[..., 3])) * 0.25
    e1r = (e[..., 0] - e[..., 2]) * 0.5
    e1i = (e[..., 3] - e[..., 1]) * 0.5

    import ml_dtypes

    def chunk(m):  # (GX, GY) -> (YCN, XC, 128, YCS)
        return np.ascontiguousarray(
            m.reshape(XC, 128, YCN, YCS).transpose(2, 0, 1, 3)).astype(ml_dtypes.bfloat16)
    return chunk(e0), chunk(e1r), chunk(e1i), chunk(e2)


def _prep_x(x):
    """x (B, IN) f32 -> 4 pre-butterfly streams [4, GX, B] bf16 (transposed)."""
    import ml_dtypes
    xT = np.ascontiguousarray(np.asarray(x, dtype=np.float32).T)  # [IN, B]
    xb = xT.reshape(GX, BLK, B)
    x0, x1, x2, x3 = xb[:, 0], xb[:, 1], xb[:, 2], xb[:, 3]
    xs = np.stack([x0 + x2, x1 + x3, x0 - x2, x3 - x1])  # s02, s13, X1r, X1i
    return xs.astype(ml_dtypes.bfloat16)


def _in_maps(x, eigens):
    e0, e1r, e1i, e2 = _prep_eigens(np.asarray(eigens))
    xs = _prep_x(x)
    return [
        {"xs": np.ascontiguousarray(xs[:, :, c * BS:(c + 1) * BS]),
         "e0": e0, "e1r": e1r, "e1i": e1i, "e2": e2}
        for c in range(NCORES)
    ]


def _combine(ys_list):
    """Per-core [4, BS, GY] bf16 Y-streams -> full (B, OUT) f32 output."""
    ys = np.concatenate([np.asarray(y).astype(np.float32) for y in ys_list],
                        axis=1)  # [4, B, GY]: Y0, Y2, Y1r, Y1i
    a = ys[0] + ys[1]
    b = ys[0] - ys[1]
    out = np.empty((B, GY, BLK), dtype=np.float32)
    out[..., 0] = a + ys[2]
    out[..., 1] = b - ys[3]
    out[..., 2] = a - ys[2]
    out[..., 3] = b + ys[3]
    return out.reshape(B, OUT)


def kernel(x, eigens):
    from concourse.bass_utils import run_bass_kernel_spmd

    if "nc" not in _cache:
        _cache["nc"] = _build_nc()
    res = run_bass_kernel_spmd(_cache["nc"], _in_maps(x, eigens),
                               core_ids=list(range(NCORES)))
    return _combine([r["ys"] for r in res.results])
